# revision 19
# baseline (speedup 1.0000x reference)
"""Trainium2 Bass kernel for nn_BrixiaCustomLoss (raw Bass, 8-core SPMD).

Reference computation (per full input):
  outputs: [B, 6, 4] f32 logits, labels: [B, 6] int32 in 0..3
  bce      = softplus(x) - x * onehot(label)        (elementwise)
  loss_bce = sum(bce) / (B*4)
  probs    = softmax(x, -1)
  labels_predicted = argmax(probs)   (f32)
  probs_predicted  = max(probs)
  regressed        = sum_c c * probs[..., c]
  loss_reg = sum(|regressed - label|) / B
  total    = 0.5*loss_bce + 0.5*loss_reg
  returns (total, labels_predicted, probs_predicted, regressed[:, -1])

Pure data parallel: batch is split over 8 cores; each core streams its
131072-row shard through SBUF in 8 double-buffered tiles of 128 rows per
partition. ACT does exp and log1p (softplus sum); DVE does reductions,
the tournament argmax, the softmax normalization, and the one-hot
x[label] dot products. The scalar loss needs only global partial sums,
which are finished on the host in float64 - no device collectives.

Raw Bass (explicit semaphores) is used because this container's walrus
accepts at most ONE sync-wait per TPB instruction; Tile's scheduler
emits two or more. All cross-engine waits are standalone wait_ge
instructions, and each instruction carries at most one then_inc.
"""

import os
import sys

import numpy as np

for _p in ("/opt/trn_rl_repo",):
    if _p not in sys.path and os.path.isdir(_p):
        sys.path.insert(0, _p)

import concourse.bass as bass  # noqa: E402
import concourse.mybir as mybir  # noqa: E402
from concourse.bass_utils import run_bass_kernel_spmd  # noqa: E402

USE_STT = os.environ.get("K_STT", "0") == "1"        # scalar_tensor_tensor
USE_ACT_ACCUM = os.environ.get("K_ACTACC", "0") == "1"  # activation accum_out
USE_RECIP_FAST = os.environ.get("K_RECIP", "0") == "1"  # reciprocal_approx_fast

F32 = mybir.dt.float32
I32 = mybir.dt.int32
AF = mybir.ActivationFunctionType
OP = mybir.AluOpType
AX = mybir.AxisListType

B = 1048576
N_AREAS = 6
N_CLASSES = 4
N_CORES = 8
BS = B // N_CORES          # rows per core = 131072
P = 128                    # SBUF partitions
NPP = BS // P              # rows per partition = 1024
C = 128                    # rows per partition per tile
T = NPP // C               # tiles per core = 8
CA = C * N_AREAS           # 768
CD = C * N_AREAS * N_CLASSES  # 3072


def _build_program(reps=1, internal_io=False):
    GT = reps * T
    nc = bass.Bass()

    if internal_io:
        # Timing-only build: big tensors live in device DRAM (contents are
        # irrelevant for engine timing); only the tiny acc output is external.
        x_d = nc.dram_tensor("x_int", [BS, N_AREAS * N_CLASSES], F32)
        lab_d = nc.dram_tensor("lab_int", [BS, N_AREAS], I32)
        lp_d = nc.dram_tensor("lp_int", [BS, N_AREAS], F32)
        pp_d = nc.dram_tensor("pp_int", [BS, N_AREAS], F32)
        ro_d = nc.dram_tensor("ro_int", [BS], F32)
        _seed = nc.dram_tensor("seed", [1, 4], F32, kind="ExternalInput")
    else:
        x_d = nc.dram_tensor(
            "x", [BS, N_AREAS * N_CLASSES], F32, kind="ExternalInput")
        lab_d = nc.dram_tensor("lab", [BS, N_AREAS], I32, kind="ExternalInput")
        lp_d = nc.dram_tensor("lp", [BS, N_AREAS], F32, kind="ExternalOutput")
        pp_d = nc.dram_tensor("pp", [BS, N_AREAS], F32, kind="ExternalOutput")
        ro_d = nc.dram_tensor("ro", [BS], F32, kind="ExternalOutput")
    # per-partition partial sums: [softplus(T) | absdiff(T) | xlabel(4T)]
    acc_d = nc.dram_tensor("acc", [P, 6 * T], F32, kind="ExternalOutput")

    xv = x_d[:].rearrange("(p n) d -> p n d", p=P)        # [128, NPP, 24]
    labv = lab_d[:].rearrange("(p n) a -> p n a", p=P)    # [128, NPP, 6]
    lpv = lp_d[:].rearrange("(p n) a -> p n a", p=P)
    ppv = pp_d[:].rearrange("(p n) a -> p n a", p=P)
    rov = ro_d[:].rearrange("(p n) -> p n", p=P)          # [128, NPP]

    from contextlib import ExitStack

    ctx = ExitStack()
    with ctx:
        def sb(name, shape, dt=F32):
            return ctx.enter_context(nc.sbuf_tensor(name, shape, dt))

        Xb = sb("Xb", [P, 2, CD])
        LABb = sb("LABb", [P, 2, CA], I32)
        Eb = sb("Eb", [P, 2, CD])
        LPb = sb("LPb", [P, 2, CA])
        PPb = sb("PPb", [P, 2, CA])
        ROb = sb("ROb", [P, 2, C])
        S2 = sb("S2", [P, CA * 2])
        S = sb("S", [P, CA])
        Rcp = sb("Rcp", [P, CA])
        u1 = sb("u1", [P, CA])
        W = sb("W", [P, CA])
        REG = sb("REG", [P, CA])
        m01 = sb("m01", [P, CA])
        m23 = sb("m23", [P, CA])
        EM = sb("EM", [P, CA])
        i01 = sb("i01", [P, CA])
        i23 = sb("i23", [P, CA])
        hi = sb("hi", [P, CA])
        q = sb("q", [P, CA])
        LABF = sb("LABF", [P, CA])
        dd = sb("dd", [P, CA])
        junk = sb("junk", [P, CA])
        tmp = sb("tmp", [P, CA])
        acc_sp = sb("acc_sp", [P, T])
        acc_da = sb("acc_da", [P, T])
        acc_xl = sb("acc_xl", [P, 4 * T])
        in_sem = ctx.enter_context(nc.semaphore())
        out_sem = ctx.enter_context(nc.semaphore())
        act_sem = ctx.enter_context(nc.semaphore())
        dve_sem = ctx.enter_context(nc.semaphore())
        block = ctx.enter_context(nc.Block())
        # Python-side ledgers of semaphore counts at named milestones.
        # ACT milestones are precomputed (2 instructions per tile).
        act_done = {}
        for gt in range(GT):
            act_done[("exp", gt)] = 2 * gt + 1
            act_done[("ln", gt)] = 2 * gt + 2
        dve_done = {}   # (name, t) -> dve_sem value when done

        # ---- DVE program ------------------------------------------------
        @block.vector
        def _(v):
            n = 0

            def emit(ins):
                nonlocal n
                ins.then_inc(dve_sem, 1)
                n += 1

            for gt in range(GT):
                t = gt % T
                sl = gt % 2
                X = Xb[:, sl]
                E = Eb[:, sl]
                LAB = LABb[:, sl]
                LP = LPb[:, sl]
                PP = PPb[:, sl]
                RO = ROb[:, sl]
                E4 = E.rearrange("p (c a k) -> p c a k", a=N_AREAS, k=N_CLASSES)
                X4 = X.rearrange("p (c a k) -> p c a k", a=N_AREAS, k=N_CLASSES)
                Ec = [
                    E4[:, :, :, k : k + 1].rearrange("p c a one -> p (c a one)")
                    for k in range(N_CLASSES)
                ]
                Xc = [
                    X4[:, :, :, k : k + 1].rearrange("p c a one -> p (c a one)")
                    for k in range(N_CLASSES)
                ]

                v.wait_ge(act_sem, act_done[("exp", gt)])

                S2v = S2[:].rearrange("p (c a k) -> p c a k", a=N_AREAS, k=2)
                emit(v.tensor_tensor(
                    S2v, E4[:, :, :, 0:4:2], E4[:, :, :, 1:4:2], OP.add))
                S2f = S2[:].rearrange("p (c a k) -> p c a k", a=N_AREAS, k=2)
                emit(v.tensor_tensor(
                    S[:],
                    S2f[:, :, :, 0:1].rearrange("p c a one -> p (c a one)"),
                    S2f[:, :, :, 1:2].rearrange("p c a one -> p (c a one)"),
                    OP.add))

                if USE_RECIP_FAST:
                    emit(v.reciprocal_approx_fast(out=Rcp[:], in_=S[:]))
                else:
                    emit(v.reciprocal(out=Rcp[:], in_=S[:]))

                # W = E1 + 2*E2 + 3*E3
                if USE_STT:
                    emit(v.scalar_tensor_tensor(
                        out=u1[:], in0=Ec[2], scalar=2.0, in1=Ec[1],
                        op0=OP.mult, op1=OP.add))
                    emit(v.scalar_tensor_tensor(
                        out=W[:], in0=Ec[3], scalar=3.0, in1=u1[:],
                        op0=OP.mult, op1=OP.add))
                else:
                    emit(v.tensor_scalar(
                        out=tmp[:], in0=Ec[2], scalar1=2.0, scalar2=None,
                        op0=OP.mult))
                    emit(v.tensor_tensor(u1[:], tmp[:], Ec[1], OP.add))
                    emit(v.tensor_scalar(
                        out=tmp[:], in0=Ec[3], scalar1=3.0, scalar2=None,
                        op0=OP.mult))
                    emit(v.tensor_tensor(W[:], tmp[:], u1[:], OP.add))
                emit(v.tensor_tensor(REG[:], W[:], Rcp[:], OP.mult))

                # tournament argmax on E (first-occurrence tie semantics)
                emit(v.tensor_tensor(m01[:], Ec[0], Ec[1], OP.max))
                emit(v.tensor_tensor(m23[:], Ec[2], Ec[3], OP.max))
                emit(v.tensor_tensor(EM[:], m01[:], m23[:], OP.max))
                emit(v.tensor_tensor(i01[:], Ec[1], Ec[0], OP.is_gt))
                emit(v.tensor_tensor(i23[:], Ec[3], Ec[2], OP.is_gt))
                emit(v.tensor_tensor(hi[:], m23[:], m01[:], OP.is_gt))
                # LP = i01 + hi*(2 + i23 - i01)
                if USE_STT:
                    emit(v.scalar_tensor_tensor(
                        out=q[:], in0=i23[:], scalar=2.0, in1=i01[:],
                        op0=OP.add, op1=OP.subtract))
                else:
                    emit(v.tensor_scalar(
                        out=tmp[:], in0=i23[:], scalar1=2.0, scalar2=None,
                        op0=OP.add))
                    emit(v.tensor_tensor(q[:], tmp[:], i01[:], OP.subtract))
                emit(v.tensor_tensor(tmp[:], q[:], hi[:], OP.mult))
                if gt >= 2:
                    v.wait_ge(out_sem, 16 * (3 * (gt - 2) + 1))  # LP slot free
                emit(v.tensor_tensor(LP[:], tmp[:], i01[:], OP.add))
                dve_done[("lp", gt)] = n

                if gt >= 2:
                    v.wait_ge(out_sem, 16 * (3 * (gt - 2) + 2))  # PP slot free
                emit(v.tensor_tensor(PP[:], EM[:], Rcp[:], OP.mult))
                dve_done[("pp", gt)] = n

                # labels ready (both input DMAs of this tile done)
                v.wait_ge(in_sem, 32 * (gt + 1))
                emit(v.tensor_copy(LABF[:], LAB[:]))
                dve_done[("labf", gt)] = n
                emit(v.tensor_tensor(dd[:], REG[:], LABF[:], OP.subtract))
                emit(v.tensor_reduce(
                    out=acc_da[:, t : t + 1], in_=dd[:], axis=AX.X, op=OP.add,
                    apply_absolute_value=True))

                # x[label] partial sums
                for k in range(N_CLASSES):
                    col = acc_xl[:, 4 * t + k : 4 * t + k + 1]
                    if USE_STT:
                        emit(v.scalar_tensor_tensor(
                            out=junk[:], in0=LABF[:], scalar=float(k),
                            in1=Xc[k], op0=OP.is_equal, op1=OP.mult,
                            accum_out=col))
                    else:
                        emit(v.tensor_scalar(
                            out=tmp[:], in0=LABF[:], scalar1=float(k),
                            scalar2=None, op0=OP.is_equal))
                        emit(v.tensor_tensor(junk[:], tmp[:], Xc[k], OP.mult))
                        emit(v.tensor_reduce(
                            out=col, in_=junk[:], axis=AX.X, op=OP.add))
                dve_done[("xl", gt)] = n

                # last-area regressed -> dense
                REG3 = REG[:].rearrange("p (c a) -> p c a", a=N_AREAS)
                if gt >= 2:
                    v.wait_ge(out_sem, 16 * (3 * (gt - 2) + 3))  # RO slot free
                emit(v.tensor_copy(
                    RO[:],
                    REG3[:, :, N_AREAS - 1 : N_AREAS].rearrange(
                        "p c one -> p (c one)")))
                dve_done[("ro", gt)] = n

                if not USE_ACT_ACCUM:
                    # softplus partial sum from the ACT Ln dump (on X slot)
                    v.wait_ge(act_sem, act_done[("ln", gt)])
                    emit(v.tensor_reduce(
                        out=acc_sp[:, t : t + 1], in_=X[:], axis=AX.X,
                        op=OP.add))
                    dve_done[("sp", gt)] = n
                dve_done[("end", gt)] = n

        # ---- ACT program ------------------------------------------------
        @block.scalar
        def _(s):
            for gt in range(GT):
                t = gt % T
                sl = gt % 2
                X = Xb[:, sl]
                E = Eb[:, sl]
                s.wait_ge(in_sem, 32 * gt + 16)  # X ready
                if gt >= 2:
                    # E slot free: all DVE reads of E(gt-2) done
                    s.wait_ge(dve_sem, dve_done[("end", gt - 2)])
                s.activation(E[:], X[:], AF.Exp).then_inc(act_sem, 1)

                # softplus: ln(E + 1) dumped onto X (X fully consumed by xl)
                s.wait_ge(dve_sem, dve_done[("xl", gt)])
                if USE_ACT_ACCUM:
                    s.activation(
                        X[:], E[:], AF.Ln, bias=1.0, scale=1.0,
                        accum_out=acc_sp[:, t : t + 1],
                    ).then_inc(act_sem, 1)
                else:
                    s.activation(
                        X[:], E[:], AF.Ln, bias=1.0, scale=1.0
                    ).then_inc(act_sem, 1)

        # ---- DMA program (SP sequencer, HWDGE) --------------------------
        @block.sync
        def _(sy):
            def dma_in(gt):
                t = gt % T
                sl = gt % 2
                rs = slice(t * C, (t + 1) * C)
                if gt >= 2:
                    # X slot free only after Ln(gt-2) wrote its dump there
                    sy.wait_ge(act_sem, act_done[("ln", gt - 2)])
                    if not USE_ACT_ACCUM:
                        # ... and after the DVE softplus reduce read it
                        sy.wait_ge(dve_sem, dve_done[("sp", gt - 2)])
                sy.dma_start(Xb[:, sl], xv[:, rs, :]).then_inc(in_sem, 16)
                if gt >= 2:
                    sy.wait_ge(dve_sem, dve_done[("labf", gt - 2)])
                sy.dma_start(LABb[:, sl], labv[:, rs, :]).then_inc(in_sem, 16)

            dma_in(0)
            if GT > 1:
                dma_in(1)
            for gt in range(GT):
                t = gt % T
                sl = gt % 2
                rs = slice(t * C, (t + 1) * C)
                sy.wait_ge(dve_sem, dve_done[("lp", gt)])
                sy.dma_start(lpv[:, rs, :], LPb[:, sl]).then_inc(out_sem, 16)
                sy.wait_ge(dve_sem, dve_done[("pp", gt)])
                sy.dma_start(ppv[:, rs, :], PPb[:, sl]).then_inc(out_sem, 16)
                sy.wait_ge(dve_sem, dve_done[("ro", gt)])
                sy.dma_start(rov[:, rs], ROb[:, sl]).then_inc(out_sem, 16)
                if gt + 2 < GT:
                    dma_in(gt + 2)

            # partial-sum outputs
            if USE_ACT_ACCUM:
                sy.wait_ge(act_sem, act_done[("ln", GT - 1)])
            else:
                sy.wait_ge(dve_sem, dve_done[("sp", GT - 1)])
            sy.dma_start(acc_d[:, 0:T], acc_sp[:]).then_inc(out_sem, 16)
            sy.wait_ge(dve_sem, dve_done[("end", GT - 1)])
            sy.dma_start(acc_d[:, T : 2 * T], acc_da[:]).then_inc(out_sem, 16)
            sy.dma_start(acc_d[:, 2 * T : 6 * T], acc_xl[:]).then_inc(
                out_sem, 16)

    return nc


_CACHED = {}


def _get_program(reps=1, internal_io=False):
    key = (reps, internal_io)
    if key not in _CACHED:
        _CACHED[key] = _build_program(reps, internal_io)
    return _CACHED[key]


def run_timing(reps):
    """Run the internal-IO timing build; returns wall seconds (best of 1)."""
    import time as _time

    nc = _get_program(reps, internal_io=True)
    seed = np.zeros((1, 4), dtype=np.float32)
    in_maps = [{"seed": seed} for _ in range(N_CORES)]
    # warm-up/compile
    run_bass_kernel_spmd(nc, in_maps, core_ids=list(range(N_CORES)))
    best = float("inf")
    for _ in range(5):
        t0 = _time.time()
        run_bass_kernel_spmd(nc, in_maps, core_ids=list(range(N_CORES)))
        best = min(best, _time.time() - t0)
    return best


def run(outputs, labels, trace=False, reps=1):
    nc = _get_program(reps)
    x = np.ascontiguousarray(np.asarray(outputs).reshape(B, N_AREAS * N_CLASSES))
    lab = np.ascontiguousarray(np.asarray(labels).reshape(B, N_AREAS))
    in_maps = [
        {
            "x": x[c * BS : (c + 1) * BS],
            "lab": lab[c * BS : (c + 1) * BS],
        }
        for c in range(N_CORES)
    ]
    res = run_bass_kernel_spmd(nc, in_maps, core_ids=list(range(N_CORES)), trace=trace)
    outs = res.results

    lp = np.concatenate([outs[c]["lp"] for c in range(N_CORES)], axis=0)
    pp = np.concatenate([outs[c]["pp"] for c in range(N_CORES)], axis=0)
    ro = np.concatenate([outs[c]["ro"] for c in range(N_CORES)], axis=0)

    sp_sum = 0.0
    xl_sum = 0.0
    dabs_sum = 0.0
    for c in range(N_CORES):
        acc = outs[c]["acc"].astype(np.float64)
        sp_sum += acc[:, 0:T].sum()
        dabs_sum += acc[:, T : 2 * T].sum()
        xl_sum += acc[:, 2 * T : 6 * T].sum()

    loss_bce = (sp_sum - xl_sum) / (B * N_CLASSES)
    loss_reg = dabs_sum / B
    total = np.float32(0.5 * loss_bce + 0.5 * loss_reg)
    return (total, lp, pp, ro), res


def kernel(outputs, labels):
    (total, lp, pp, ro), _ = run(outputs, labels)
    return total, lp, pp, ro


# revision 25
# speedup vs baseline: 3.4817x; 3.4817x over previous
"""Trainium2 Bass kernel for nn_BrixiaCustomLoss (raw Bass, 8-core SPMD).

Reference computation (per full input):
  outputs: [B, 6, 4] f32 logits, labels: [B, 6] int32 in 0..3
  bce      = softplus(x) - x * onehot(label)        (elementwise)
  loss_bce = sum(bce) / (B*4)
  probs    = softmax(x, -1)
  labels_predicted = argmax(probs)   (f32)
  probs_predicted  = max(probs)
  regressed        = sum_c c * probs[..., c]
  loss_reg = sum(|regressed - label|) / B
  total    = 0.5*loss_bce + 0.5*loss_reg
  returns (total, labels_predicted, probs_predicted, regressed[:, -1])

Pure data parallel: batch split over 8 cores. In this environment each
dispatched instruction costs ~33us regardless of size (host-mediated
execution), so the kernel is designed to MINIMIZE INSTRUCTION COUNT:

- inputs are host-packed to one [rows, 30] f32 array (24 logits + 6
  int32 labels bit-cast) so each tile needs ONE input DMA; outputs are
  packed [rows, 13] (6 lp + 6 pp + 1 ro) for ONE output DMA.
- per-class-group reductions use single tensor_tensor_scan instructions
  with a periodic reset mask ([0,1,1,1]): running-sum (S), running-max
  (EM with inclusive prefix maxes), and a Horner-style weighted sum
  W = E1+2*E2+3*E3 via coefficients [0, 1/2, 2/3, 3].
- argmax with exact first-occurrence ties: LP = sum_c [prefix_max_c < max]
  (one is_lt + one 4-group reduce).
- x[label] global sum: one one-hot compare vs an iota pattern + one
  fused scalar_tensor_tensor multiply with accumulate per tile.
- softplus global sum: ACT Ln(E + 1) with accum_out.

The scalar loss needs only per-partition partial sums finished on the
host in float64 - no collectives. Raw Bass with explicit semaphores is
required: this container's walrus accepts at most ONE sync-wait per
instruction, which Tile's scheduler does not respect.
"""

import os
import sys

import numpy as np

for _p in ("/opt/trn_rl_repo",):
    if _p not in sys.path and os.path.isdir(_p):
        sys.path.insert(0, _p)

import concourse.bass as bass  # noqa: E402
import concourse.mybir as mybir  # noqa: E402
from concourse.bass_utils import run_bass_kernel_spmd  # noqa: E402

F32 = mybir.dt.float32
BF16 = mybir.dt.bfloat16
I32 = mybir.dt.int32
AF = mybir.ActivationFunctionType
OP = mybir.AluOpType
AX = mybir.AxisListType

B = 1048576
N_AREAS = 6
N_CLASSES = 4
N_CORES = 8
BS = B // N_CORES          # rows per core = 131072
P = 128                    # SBUF partitions
NPP = BS // P              # rows per partition = 1024
C = 256                    # rows per partition per tile
T = NPP // C               # tiles per core = 4
CA = C * N_AREAS           # 1536
CD = C * N_AREAS * N_CLASSES  # 6144
NIN = 30                   # packed input row: 24 x + 6 labels
NOUT = 18                  # packed output row: 6 lp + 6 pp + 6 reg


def _build_program(reps=1, internal_io=False):
    GT = reps * T
    nc = bass.Bass()

    if internal_io:
        in_d = nc.dram_tensor("in_int", [BS, NIN], F32)
        out_d = nc.dram_tensor("out_int", [BS, NOUT], F32)
        _seed = nc.dram_tensor("seed", [1, 4], F32, kind="ExternalInput")
    else:
        in_d = nc.dram_tensor("inp", [BS, NIN], F32, kind="ExternalInput")
        out_d = nc.dram_tensor("outp", [BS, NOUT], F32, kind="ExternalOutput")
    # per-partition partial sums: [softplus(T) | absdiff(T) | xlabel(T)]
    acc_d = nc.dram_tensor("acc", [P, 3 * T], F32, kind="ExternalOutput")

    inv = in_d[:].rearrange("(p n) d -> p n d", p=P)     # [128, NPP, 30]
    outv = out_d[:].rearrange("(p n) d -> p n d", p=P)   # [128, NPP, 13]

    from contextlib import ExitStack

    ctx = ExitStack()
    with ctx:
        def sb(name, shape, dt=F32):
            return ctx.enter_context(nc.sbuf_tensor(name, shape, dt))

        IN = sb("IN", [P, C, NIN])          # packed input tile
        E = sb("E", [P, CD])                # exp(x)
        OUT = sb("OUT", [P, C, NOUT])       # packed output tile
        A = sb("A", [P, CD])                # scratch (scan outputs etc.)
        Bt = sb("Bt", [P, CD])              # scratch
        SPD = sb("SPD", [P, CD], BF16)      # Ln dump (only accum matters)
        K0111 = sb("K0111", [P, CD], BF16)  # periodic reset mask 0,1,1,1
        KW = sb("KW", [P, CD])              # periodic 0, 1/2, 2/3, 3
        IOTA = sb("IOTA", [P, N_CLASSES])   # 0,1,2,3
        Stmp = sb("Stmp", [P, CA])
        Rcp = sb("Rcp", [P, CA])
        ddt = sb("ddt", [P, CA])
        acc = sb("acc_t", [P, 3 * T])
        in_sem = ctx.enter_context(nc.semaphore())
        out_sem = ctx.enter_context(nc.semaphore())
        act_sem = ctx.enter_context(nc.semaphore())
        dve_sem = ctx.enter_context(nc.semaphore())
        block = ctx.enter_context(nc.Block())

        # views
        A4 = A[:].rearrange("p (c a k) -> p c a k", a=N_AREAS, k=N_CLASSES)
        B4 = Bt[:].rearrange("p (c a k) -> p c a k", a=N_AREAS, k=N_CLASSES)
        E2 = E[:]
        A2 = A[:]
        B2 = Bt[:]
        A3 = A[:].rearrange("p (c d) -> p c d", d=N_AREAS * N_CLASSES)
        B3 = Bt[:].rearrange("p (c d) -> p c d", d=N_AREAS * N_CLASSES)
        E3 = E[:].rearrange("p (c d) -> p c d", d=N_AREAS * N_CLASSES)
        K2 = K0111[:]
        KW2 = KW[:]
        Rcp3 = Rcp[:].rearrange("p (c a) -> p c a", a=N_AREAS)
        REG3 = OUT[:, :, 12:18]
        dd3 = ddt[:].rearrange("p (c a) -> p c a", a=N_AREAS)

        def sl4(t4, k):
            # [P, C, 6] view of class-k lane of a [P, C, 6, 4] view
            return t4[:, :, :, k : k + 1].rearrange("p c a one -> p c (a one)")

        act_done = {}
        for gt in range(GT):
            act_done[("exp", gt)] = 2 * gt + 1
            act_done[("ln", gt)] = 2 * gt + 2
        dve_done = {}

        # ---- DVE program ------------------------------------------------
        @block.vector
        def _(v):
            n = 0

            def emit(ins):
                nonlocal n
                ins.then_inc(dve_sem, 1)
                n += 1

            # one-time constants (DVE-only consumers -> no cross-engine sync)
            K4 = K0111[:].rearrange(
                "p (c a k) -> p c a k", a=N_AREAS, k=N_CLASSES)
            KW4 = KW[:].rearrange(
                "p (c a k) -> p c a k", a=N_AREAS, k=N_CLASSES)
            emit(v.memset(K2, 1.0))
            emit(v.memset(sl4(K4, 0), 0.0))
            emit(v.memset(sl4(KW4, 0), 0.0))
            emit(v.memset(sl4(KW4, 1), 0.5))
            emit(v.memset(sl4(KW4, 2), 2.0 / 3.0))
            emit(v.memset(sl4(KW4, 3), 3.0))
            for k in range(N_CLASSES):
                emit(v.memset(IOTA[:, k : k + 1], float(k)))

            for gt in range(GT):
                t = gt % T
                x3 = IN[:, :, 0:24]                       # [P, C, 24] f32
                lab3 = IN[:, :, 24:30].bitcast(I32)       # [P, C, 6] i32
                labB = lab3[:, :, :, None].broadcast_to([P, C, N_AREAS, 4])
                iotaB = IOTA[:, None, None, :].broadcast_to(
                    [P, C, N_AREAS, 4])
                emB = A4[:, :, :, 3:4].broadcast_to([P, C, N_AREAS, 4])
                E4 = E[:].rearrange(
                    "p (c a k) -> p c a k", a=N_AREAS, k=N_CLASSES)
                rs = slice(t * C, (t + 1) * C)

                v.wait_ge(act_sem, act_done[("exp", gt)])

                # inclusive prefix-max per 4-group (EM at lane 3)
                emit(v.tensor_tensor_scan(
                    A2, K2, E2, 0.0, op0=OP.mult, op1=OP.max))
                # group sum S + reciprocal
                emit(v.tensor_reduce(
                    out=Stmp[:], in_=E4, axis=AX.X, op=OP.add))
                emit(v.reciprocal(out=Rcp[:], in_=Stmp[:]))
                # Horner weighted sum W = E1 + 2 E2 + 3 E3 (lane 3)
                emit(v.tensor_tensor_scan(
                    B2, E2, KW2, 0.0, op0=OP.add, op1=OP.mult))
                if gt >= 1:
                    v.wait_ge(out_sem, 16 * gt)  # OUT slot free (dma done)
                emit(v.tensor_tensor(REG3, sl4(B4, 3), Rcp3, OP.mult))
                emit(v.tensor_tensor(
                    OUT[:, :, 6:12], sl4(A4, 3), Rcp3, OP.mult))  # pp
                # ltm = [prefix_max < max]; lp = group sum of ltm
                emit(v.tensor_tensor(B4, A4, emB, OP.is_lt))
                emit(v.tensor_reduce(
                    out=OUT[:, :, 0:6], in_=B4, axis=AX.X, op=OP.add))
                dve_done[("lpred", gt)] = n
                # x[label] partial sum: one-hot then fused mul-mul-accum
                emit(v.tensor_tensor(A4, labB, iotaB, OP.is_equal))
                emit(v.scalar_tensor_tensor(
                    out=B3, in0=A3, scalar=1.0, in1=x3,
                    op0=OP.mult, op1=OP.mult,
                    accum_out=acc[:, 2 * T + t : 2 * T + t + 1]))
                # regression loss partials (reads OUT.reg; DMA also reads it)
                emit(v.tensor_tensor(dd3, REG3, lab3, OP.subtract))
                emit(v.tensor_reduce(
                    out=acc[:, T + t : T + t + 1], in_=ddt[:], axis=AX.X,
                    op=OP.add, apply_absolute_value=True))
                dve_done[("dd", gt)] = n

        # ---- ACT program ------------------------------------------------
        @block.scalar
        def _(s):
            for gt in range(GT):
                t = gt % T
                s.wait_ge(in_sem, 16 * (gt + 1))  # IN ready
                s.activation(E3, IN[:, :, 0:24], AF.Exp).then_inc(act_sem, 1)
                s.activation(
                    SPD[:], E2, AF.Ln, bias=1.0, scale=1.0,
                    accum_out=acc[:, t : t + 1],
                ).then_inc(act_sem, 1)

        # ---- DMA program (SP sequencer, HWDGE) --------------------------
        @block.sync
        def _(sy):
            sy.dma_start(IN[:], inv[:, 0:C, :]).then_inc(in_sem, 16)
            for gt in range(GT):
                t = gt % T
                rs = slice(t * C, (t + 1) * C)
                sy.wait_ge(dve_sem, dve_done[("lpred", gt)])
                sy.dma_start(outv[:, rs, :], OUT[:]).then_inc(out_sem, 16)
                if gt + 1 < GT:
                    t2 = (gt + 1) % T
                    rs2 = slice(t2 * C, (t2 + 1) * C)
                    sy.wait_ge(dve_sem, dve_done[("dd", gt)])
                    sy.dma_start(IN[:], inv[:, rs2, :]).then_inc(in_sem, 16)
            sy.wait_ge(dve_sem, dve_done[("dd", GT - 1)])
            sy.wait_ge(act_sem, act_done[("ln", GT - 1)])
            sy.dma_start(acc_d[:], acc[:]).then_inc(out_sem, 16)

    return nc


_CACHED = {}


def _get_program(reps=1, internal_io=False):
    key = (reps, internal_io)
    if key not in _CACHED:
        _CACHED[key] = _build_program(reps, internal_io)
    return _CACHED[key]


def run_timing(reps):
    """Run the internal-IO timing build; returns best wall seconds."""
    import time as _time

    nc = _get_program(reps, internal_io=True)
    seed = np.zeros((1, 4), dtype=np.float32)
    in_maps = [{"seed": seed} for _ in range(N_CORES)]
    run_bass_kernel_spmd(nc, in_maps, core_ids=list(range(N_CORES)))
    best = float("inf")
    for _ in range(5):
        t0 = _time.time()
        run_bass_kernel_spmd(nc, in_maps, core_ids=list(range(N_CORES)))
        best = min(best, _time.time() - t0)
    return best


def run(outputs, labels, trace=False, reps=1):
    nc = _get_program(reps)
    x = np.asarray(outputs).reshape(B, N_AREAS * N_CLASSES)
    lab = np.asarray(labels).reshape(B, N_AREAS)
    packed = np.empty((B, NIN), dtype=np.float32)
    packed[:, 0:24] = x
    packed[:, 24:30] = lab.view(np.float32)
    in_maps = [{"inp": packed[c * BS : (c + 1) * BS]} for c in range(N_CORES)]
    res = run_bass_kernel_spmd(nc, in_maps, core_ids=list(range(N_CORES)), trace=trace)
    outs = res.results

    outp = np.concatenate([outs[c]["outp"] for c in range(N_CORES)], axis=0)
    lp = np.ascontiguousarray(outp[:, 0:6])
    pp = np.ascontiguousarray(outp[:, 6:12])
    ro = np.ascontiguousarray(outp[:, 17])

    sp_sum = 0.0
    xl_sum = 0.0
    dabs_sum = 0.0
    for c in range(N_CORES):
        a = outs[c]["acc"].astype(np.float64)
        sp_sum += a[:, 0:T].sum()
        dabs_sum += a[:, T : 2 * T].sum()
        xl_sum += a[:, 2 * T : 3 * T].sum()

    loss_bce = (sp_sum - xl_sum) / (B * N_CLASSES)
    loss_reg = dabs_sum / B
    total = np.float32(0.5 * loss_bce + 0.5 * loss_reg)
    return (total, lp, pp, ro), res


def kernel(outputs, labels):
    (total, lp, pp, ro), _ = run(outputs, labels)
    return total, lp, pp, ro
